# revision 7
# baseline (speedup 1.0000x reference)
"""ChebyNet (K=3, 2 layers) forward on 8 Trainium2 NeuronCores.

Node-sharded dense formulation. The sparse propagation L = -D^-1/2 A D^-1/2
is computed as dense matmuls against the SBUF-resident transposed adjacency
counts AT[s, d] (fp8e4m3, exact small ints). All four propagation hops run
"transposed": the feature tile is the stationary operand (1 LDWEIGHTS per
k-tile pair, fully hidden under >=512-row moving streams) and the adjacency
streams as the moving operand, producing feature-major [feat, dst] PSUM.

Features entering each hop are carried as compensated fp8 pairs
(hi = fp8(x), lo = fp8(x - hi)); each hop sweeps hi then lo with
MatmulPerfMode.DoubleRow (2 k-tiles per matmul at 1 cycle/row — 2x bf16),
so a compensated hop costs the same PE time as one bf16 sweep but the
AllGather between hops ships fp8: the hi piece (half the bytes) gates the
next hop's hi-sweep while the lo piece flies in its shadow.

Layer algebra (L commutes with the feature matmuls; dis-scaling is folded
into the host-prepared x-hat = dis*x so staged payloads need no extra
scaling pass):

  u0    = x_hat @ (2 W12)                      (replicated, all NP rows)
  hop A: PA^T = AT-sweep(u0)                   s1_hat = d1_hat^T + nd2 * PA^T
  AG0(s1_hat hi, lo)
  hop B: PB^T = AT-sweep(s1_hat)               h^T = relu(e0^T + ndis*PB^T + b1)
  z1_hat^T = W21^T h_hat^T, z2_hat^T = (2 W22)^T h_hat^T, hw^T = (W20-W22)^T h^T
  AG1(z2_hat hi, lo)
  hop C: PC^T = AT-sweep(z2_hat)               s2_hat = z1_hat^T + nd2 * PC^T
  AG2(s2_hat hi, lo)
  hop D: PD^T = AT-sweep(s2_hat)               out^T = hw^T + ndis*PD^T + b2

where nd2 = -dis^2 (column-broadcast), ndis = -dis, and d1_hat/e0 come from
W-stationary matmuls over the core's own x^T columns.
"""

import sys

for _p in ("/opt/trn_rl_repo", "/root/.axon_site", "/root/.axon_site/_ro/trn_rl_repo",
           "/root/.axon_site/_ro/pypackages"):
    if _p not in sys.path:
        sys.path.append(_p)

import numpy as np
import ml_dtypes

import concourse.bacc as bacc
import concourse.tile as tile
from concourse import bass, mybir
from concourse.bass_utils import run_bass_kernel_spmd
from concourse.masks import make_identity

# problem constants (hardcoded per harness contract)
N, E, IN, HID, OUT, K = 10000, 320000, 256, 256, 128, 3
CORES = 8
NP, P = 10240, 128
RPC = NP // CORES           # rows per core = 1280
MB = RPC // P               # 128-blocks per core = 10
KT = NP // P                # src k-tiles = 80
PAIRS = KT // 2             # DoubleRow k-tile pairs = 40
F = IN                      # layer-1 prop width = 256
CHK = ((0, 512), (512, 512), (1024, 256))    # dst chunks of the 1280 own rows
PIECES = ((0, 4), (4, 10))                   # AG piece block-ranges (pair aligned)
AGW = (F, OUT, OUT)                          # AG widths per round
# hi/lo sweep parts per hop (lo droppable per-hop if error budget allows)
HOP_PARTS = {"A": (0, 1), "B": (0, 1), "C": (0, 1), "D": (0, 1)}

FP8 = mybir.dt.float8e4
BF16 = mybir.dt.bfloat16
F32 = mybir.dt.float32
DR = mybir.MatmulPerfMode.DoubleRow

_STATE = {}


def _piece_pair_sets():
    """Pairs j grouped by AG piece: piece blocks [b0,b1) -> j%(MB//2) in [b0/2,b1/2)."""
    sets = []
    for b0, b1 in PIECES:
        lo, hi = b0 // 2, b1 // 2
        sets.append([j for j in range(PAIRS) if lo <= (j % (MB // 2)) < hi])
    return sets


def _build():
    nc = bacc.Bacc("TRN2", target_bir_lowering=False, debug=False, num_devices=CORES)

    at_d = nc.dram_tensor("at", [P, KT * RPC], FP8, kind="ExternalInput")
    xhT_d = nc.dram_tensor("xhT", [2, P, NP], BF16, kind="ExternalInput")
    xoT_d = nc.dram_tensor("xoT", [2, P, RPC], BF16, kind="ExternalInput")
    w1x_d = nc.dram_tensor("w1x", [K, IN, HID], BF16, kind="ExternalInput")
    w2x_d = nc.dram_tensor("w2x", [K, HID, OUT], BF16, kind="ExternalInput")
    b1f_d = nc.dram_tensor("b1f", [P, 2], F32, kind="ExternalInput")
    b2f_d = nc.dram_tensor("b2f", [P, 1], F32, kind="ExternalInput")
    ndisb_d = nc.dram_tensor("ndisb", [P, RPC], F32, kind="ExternalInput")
    nd2b_d = nc.dram_tensor("nd2b", [P, RPC], BF16, kind="ExternalInput")
    disb_d = nc.dram_tensor("disb", [P, RPC], BF16, kind="ExternalInput")
    out_d = nc.dram_tensor("outo", [RPC, OUT], F32, kind="ExternalOutput")

    psets = _piece_pair_sets()
    order_nat = list(range(PAIRS))
    order_pc = [j for s in psets for j in s]

    with tile.TileContext(nc) as tc:
        with (
            tc.tile_pool(name="res", bufs=1) as res,
            tc.tile_pool(name="wrk", bufs=1) as wrk,
            tc.tile_pool(name="pacc", bufs=1, space="PSUM") as pacc,
            tc.tile_pool(name="pterm", bufs=1, space="PSUM") as pterm,
            tc.tile_pool(name="ptr", bufs=1, space="PSUM") as ptr,
            tc.tile_pool(name="dram", bufs=1, space="DRAM") as dram,
        ):
            # ---- small loads (sync queue) ----
            w1t = [[None, None] for _ in range(K)]
            w2t = [[None, None] for _ in range(K)]
            for k in range(K):
                for ih in range(2):
                    t = res.tile([P, HID], BF16, tag=f"w1_{k}_{ih}", name=f"w1_{k}_{ih}")
                    nc.sync.dma_start(t[:], w1x_d[k, ih * P:(ih + 1) * P, :])
                    w1t[k][ih] = t
                    t2 = res.tile([P, OUT], BF16, tag=f"w2_{k}_{ih}", name=f"w2_{k}_{ih}")
                    nc.sync.dma_start(t2[:], w2x_d[k, ih * P:(ih + 1) * P, :])
                    w2t[k][ih] = t2
            b1f = res.tile([P, 2], F32, name="b1f")
            nc.sync.dma_start(b1f[:], b1f_d[:])
            b2f = res.tile([P, 1], F32, name="b2f")
            nc.sync.dma_start(b2f[:], b2f_d[:])
            ndisb = res.tile([P, RPC], F32, name="ndisb")
            nc.sync.dma_start(ndisb[:], ndisb_d[:])
            nd2b = res.tile([P, RPC], BF16, name="nd2b")
            nc.sync.dma_start(nd2b[:], nd2b_d[:])
            disb = res.tile([P, RPC], BF16, name="disb")
            nc.sync.dma_start(disb[:], disb_d[:])
            xoT = []
            for ih in range(2):
                t = res.tile([P, RPC], BF16, tag=f"xoT{ih}", name=f"xoT{ih}")
                nc.sync.dma_start(t[:], xoT_d[ih])
                xoT.append(t)

            ident = res.tile([P, P], F32, name="ident")
            make_identity(nc, ident[:])
            idb = res.tile([P, P], BF16, name="idb")
            nc.vector.tensor_copy(idb[:], ident[:])

            # ---- dummy AG: absorb one-time collective bootstrap ----
            dumi = dram.tile([P, 16], FP8, name="dumi")
            dumo = dram.tile([CORES * P, 16], FP8, name="dumo", addr_space="Shared")
            nc.sync.dma_start(dumi[:], at_d[:, 0:16])
            nc.gpsimd.collective_compute(
                "AllGather", mybir.AluOpType.bypass,
                replica_groups=[list(range(CORES))],
                ins=[dumi[:].opt()], outs=[dumo[:].opt()],
            )

            # ---- adjacency: scalar queue streams most of it, sync takes the
            # tail after the x^T chunks. hop A's pair order chases arrival.
            at_res = res.tile([P, KT, RPC], FP8, name="at_res")
            CH = 8
            ktpc = KT // CH
            at_r = at_d.ap().rearrange("p (k d) -> p k d", k=KT)
            for ch in range(6):
                nc.scalar.dma_start(at_res[:, ch * ktpc:(ch + 1) * ktpc, :],
                                    at_r[:, ch * ktpc:(ch + 1) * ktpc, :])

            # ---- replicated u0 = x_hat @ (2 W12) -> fp8 hi/lo ----
            u_hi = res.tile([P, KT, F], FP8, name="u_hi")
            u_lo = res.tile([P, KT, F], FP8, name="u_lo")
            HCH = 16
            hw_cols = NP // HCH  # 640
            for hc in range(HCH):
                xc = [wrk.tile([P, hw_cols], BF16, tag=f"xc{ih}", bufs=2,
                               name=f"xc{hc}_{ih}") for ih in range(2)]
                for ih in range(2):
                    nc.sync.dma_start(xc[ih][:],
                                      xhT_d[ih, :, hc * hw_cols:(hc + 1) * hw_cols])
                for m in range(hw_cols // P):
                    kt = hc * (hw_cols // P) + m
                    dp = pterm.tile([P, F], F32, tag="pt", bufs=2, name=f"d2_{kt}")
                    nc.tensor.matmul(dp[:], xc[0][:, m * P:(m + 1) * P], w1t[2][0][:],
                                     start=True, stop=False)
                    nc.tensor.matmul(dp[:], xc[1][:, m * P:(m + 1) * P], w1t[2][1][:],
                                     start=False, stop=True)
                    nc.vector.tensor_copy(u_hi[:, kt, :], dp[:])
                    nc.vector.tensor_sub(u_lo[:, kt, :], dp[:], u_hi[:, kt, :])

            # at tail on sync after the x^T stream
            for ch in range(6, CH):
                nc.sync.dma_start(at_res[:, ch * ktpc:(ch + 1) * ktpc, :],
                                  at_r[:, ch * ktpc:(ch + 1) * ktpc, :])

            # ---- d1_hat^T, e0^T (feature-major, W-stationary, own rows) ----
            CH5 = tuple((i * 256, 256) for i in range(5))
            d1T = [res.tile([P, RPC], BF16, tag=f"d1T{c}", name=f"d1T{c}") for c in range(2)]
            e0T = [res.tile([P, RPC], BF16, tag=f"e0T{c}", name=f"e0T{c}") for c in range(2)]
            for dst, wk, scaled in ((d1T, 1, True), (e0T, 0, False)):
                for hh in range(2):
                    for off, w in CH5:
                        pd = pterm.tile([P, F], F32, tag="pt", bufs=2,
                                        name=f"f{wk}_{hh}_{off}")
                        for ih in range(2):
                            nc.tensor.matmul(
                                pd[:, :w], w1t[wk][ih][:, hh * P:(hh + 1) * P],
                                xoT[ih][:, off:off + w],
                                start=(ih == 0), stop=(ih == 1))
                        if scaled:
                            nc.vector.tensor_mul(dst[hh][:, off:off + w], pd[:, :w],
                                                 disb[:, off:off + w])
                        else:
                            nc.vector.tensor_copy(dst[hh][:, off:off + w], pd[:, :w])

            # ---- sweep machinery ----
            def sweep(parts, order, nchalf, tagn):
                pas = []
                for c in range(nchalf):
                    row = []
                    for ci, (off, w) in enumerate(CHK):
                        if w == 512:
                            row.append(pacc.tile([P, 512], F32, tag=f"pa{c}_{ci}",
                                                 name=f"{tagn}_{c}_{ci}"))
                        else:
                            row.append(pterm.tile([P, F], F32, tag="pt", bufs=2,
                                                  name=f"{tagn}_{c}_{ci}"))
                    pas.append(row)
                for pi, ut in enumerate(parts):
                    for jn, j in enumerate(order):
                        first = (pi == 0 and jn == 0)
                        last = (pi == len(parts) - 1 and jn == len(order) - 1)
                        for c in range(nchalf):
                            lhsT = ut[:, 2 * j:2 * j + 2, c * P:c * P + P]
                            for ci, (off, w) in enumerate(CHK):
                                nc.tensor.matmul(
                                    pas[c][ci][:, :w], lhsT,
                                    at_res[:, 2 * j:2 * j + 2, off:off + w],
                                    start=first, stop=last, perf_mode=DR)
                return pas

            # staging: node-major fp8 hi/lo payloads
            sgN_hi = res.tile([P, MB, F], FP8, name="sgN_hi")
            sgN_lo = res.tile([P, MB, F], FP8, name="sgN_lo")
            sA = [wrk.tile([P, RPC], BF16, tag=f"sA{c}", name=f"sA{c}") for c in range(2)]

            ag_in = [[[dram.tile([(b1 - b0) * P, AGW[r]], FP8, name=f"agi{r}_{part}_{pc}")
                       for pc, (b0, b1) in enumerate(PIECES)]
                      for part in range(2)] for r in range(3)]
            ag_out = [[[dram.tile([CORES * (b1 - b0) * P, AGW[r]], FP8,
                                  name=f"ago{r}_{part}_{pc}", addr_space="Shared")
                        for pc, (b0, b1) in enumerate(PIECES)]
                       for part in range(2)] for r in range(3)]

            def stage_block(mb, c, src):
                tp = ptr.tile([P, P], BF16, tag="ptr", bufs=2, name=f"tp_{mb}_{c}")
                nc.tensor.transpose(tp[:], src, idb[:])
                nc.vector.tensor_copy(sgN_hi[:, mb, c * P:c * P + P], tp[:])
                nc.vector.tensor_sub(sgN_lo[:, mb, c * P:c * P + P], tp[:],
                                     sgN_hi[:, mb, c * P:c * P + P])

            def emit_ag(r, part, pc):
                b0, b1 = PIECES[pc]
                sg = sgN_hi if part == 0 else sgN_lo
                nc.sync.dma_start(
                    ag_in[r][part][pc][:].rearrange("(b p) f -> p b f", p=P),
                    sg[:, b0:b1, :AGW[r]])
                nc.gpsimd.collective_compute(
                    "AllGather", mybir.AluOpType.bypass,
                    replica_groups=[list(range(CORES))],
                    ins=[ag_in[r][part][pc][:].opt()],
                    outs=[ag_out[r][part][pc][:].opt()],
                )

            def reload(r, part, pc):
                b0, b1 = PIECES[pc]
                nb = b1 - b0
                ut = u_hi if part == 0 else u_lo
                for c in range(CORES):
                    src = ag_out[r][part][pc][c * nb * P:(c + 1) * nb * P, :] \
                        .rearrange("(b p) f -> p b f", p=P)
                    nc.scalar.dma_start(ut[:, c * MB + b0:c * MB + b0 + nb, :AGW[r]], src)

            def stage_round(r, nchalf, zsrc):
                """Transpose zsrc (feature-major bf16 [c][P,RPC]) into node-major
                fp8 hi/lo staging tiles; fire AG pieces hi-first."""
                for pc, (b0, b1) in enumerate(PIECES):
                    for mb in range(b0, b1):
                        for c in range(nchalf):
                            stage_block(mb, c, zsrc[c][:, mb * P:(mb + 1) * P])
                    emit_ag(r, 0, pc)
                for pc in range(len(PIECES)):
                    emit_ag(r, 1, pc)
                for part in range(2):
                    for pc in range(len(PIECES)):
                        reload(r, part, pc)

            # ================= hop A =================
            parts_A = [(u_hi, u_lo)[i] for i in HOP_PARTS["A"]]
            pa = sweep(parts_A, order_nat, 2, "swA")
            # post: s1_hat^T = d1_hat^T + nd2 * PA^T
            for c in range(2):
                for ci, (off, w) in enumerate(CHK):
                    nc.vector.tensor_mul(sA[c][:, off:off + w], pa[c][ci][:, :w],
                                         nd2b[:, off:off + w])
                    nc.vector.tensor_add(sA[c][:, off:off + w], sA[c][:, off:off + w],
                                         d1T[c][:, off:off + w])
            stage_round(0, 2, sA)

            # ================= hop B =================
            parts_B = [(u_hi, u_lo)[i] for i in HOP_PARTS["B"]]
            pa = sweep(parts_B, order_pc, 2, "swB")
            # post: h^T = relu(e0^T + ndis*PB^T + b1); hb bf16, hhb = h*dis
            hb = [res.tile([P, RPC], BF16, tag=f"hb{c}", name=f"hb{c}") for c in range(2)]
            hhb = [res.tile([P, RPC], BF16, tag=f"hhb{c}", name=f"hhb{c}")
                   for c in range(2)]
            for c in range(2):
                for ci, (off, w) in enumerate(CHK):
                    t = wrk.tile([P, 512], F32, tag="t32", bufs=2, name=f"t32_{c}_{ci}")
                    nc.vector.tensor_mul(t[:, :w], pa[c][ci][:, :w], ndisb[:, off:off + w])
                    nc.vector.tensor_add(t[:, :w], t[:, :w], e0T[c][:, off:off + w])
                    nc.vector.tensor_scalar_add(t[:, :w], t[:, :w], b1f[:, c:c + 1])
                    nc.vector.tensor_scalar_max(hb[c][:, off:off + w], t[:, :w], 0.0)
                    nc.gpsimd.tensor_mul(hhb[c][:, off:off + w], hb[c][:, off:off + w],
                                         disb[:, off:off + w])

            # layer-2 feature matmuls from h^T (W2 stationary, h moving)
            z1T = res.tile([P, RPC], BF16, tag="z1T", name="z1T")
            hwT = res.tile([P, RPC], BF16, tag="hwT", name="hwT")
            for wk, mov, dst in ((1, hhb, z1T), (2, hhb, sA[0]), (0, hb, hwT)):
                for off, w in CH5:
                    pz = pterm.tile([P, F], F32, tag="pt", bufs=2, name=f"z{wk}_{off}")
                    for ih in range(2):
                        nc.tensor.matmul(pz[:, :w], w2t[wk][ih][:],
                                         mov[ih][:, off:off + w],
                                         start=(ih == 0), stop=(ih == 1))
                    nc.vector.tensor_copy(dst[:, off:off + w], pz[:, :w])

            stage_round(1, 1, sA)

            # ================= hop C =================
            parts_C = [(u_hi, u_lo)[i] for i in HOP_PARTS["C"]]
            pa = sweep(parts_C, order_pc, 1, "swC")
            # post: s2_hat^T = z1_hat^T + nd2 * PC^T
            for ci, (off, w) in enumerate(CHK):
                nc.vector.tensor_mul(sA[0][:, off:off + w], pa[0][ci][:, :w],
                                     nd2b[:, off:off + w])
                nc.vector.tensor_add(sA[0][:, off:off + w], sA[0][:, off:off + w],
                                     z1T[:, off:off + w])
            stage_round(2, 1, sA)

            # ================= hop D =================
            parts_D = [(u_hi, u_lo)[i] for i in HOP_PARTS["D"]]
            pa = sweep(parts_D, order_pc, 1, "swD")
            # post: out^T = hw^T + ndis*PD^T + b2 -> transpose -> DMA out
            for ci, (off, w) in enumerate(CHK):
                t = wrk.tile([P, 512], F32, tag="t32", bufs=2, name=f"to_{ci}")
                nc.vector.tensor_mul(t[:, :w], pa[0][ci][:, :w], ndisb[:, off:off + w])
                nc.vector.tensor_add(t[:, :w], t[:, :w], hwT[:, off:off + w])
                nc.vector.tensor_scalar_add(t[:, :w], t[:, :w], b2f[:, 0:1])
                for mi in range(w // P):
                    mb = off // P + mi
                    tb = wrk.tile([P, P], BF16, tag="tb", bufs=2, name=f"tb_{mb}")
                    nc.vector.tensor_copy(tb[:], t[:, mi * P:(mi + 1) * P])
                    tf = ptr.tile([P, P], BF16, tag="ptr", bufs=2, name=f"tf_{mb}")
                    nc.tensor.transpose(tf[:], tb[:], idb[:])
                    ob = wrk.tile([P, OUT], F32, tag="ob", bufs=2, name=f"ob_{mb}")
                    nc.vector.tensor_copy(ob[:], tf[:])
                    nc.sync.dma_start(out_d[mb * P:(mb + 1) * P, :], ob[:])

    nc.compile()
    return nc


def _prepare_inputs(x, edge, W1, b1, W2, b2):
    x = np.asarray(x, np.float32)
    edge = np.asarray(edge)
    W1 = np.asarray(W1, np.float32)
    b1 = np.asarray(b1, np.float32)
    W2 = np.asarray(W2, np.float32)
    b2 = np.asarray(b2, np.float32)
    src = edge[0].astype(np.int64)
    dst = edge[1].astype(np.int64)

    deg = np.bincount(dst, minlength=N).astype(np.float32)
    dis = np.where(deg > 0, 1.0 / np.sqrt(np.maximum(deg, 1.0)), 0.0).astype(np.float32)

    # dense transposed adjacency counts AT[s, d] (exact in fp8)
    flat = src * NP + dst
    uniq, cnt = np.unique(flat, return_counts=True)
    at8 = np.zeros(NP * NP, dtype=ml_dtypes.float8_e4m3)
    at8[uniq] = cnt.astype(ml_dtypes.float8_e4m3)
    at8 = at8.reshape(NP, NP)

    dis_pad = np.zeros(NP, np.float32)
    dis_pad[:N] = dis
    x_pad = np.zeros((NP, F), np.float32)
    x_pad[:N] = x
    xh_pad = x_pad * dis_pad[:, None]

    w1x = np.stack([W1[0] - W1[2], W1[1], 2.0 * W1[2]]).astype(ml_dtypes.bfloat16)
    w2x = np.stack([W2[0] - W2[2], W2[1], 2.0 * W2[2]]).astype(ml_dtypes.bfloat16)
    b1f = np.ascontiguousarray(b1.reshape(2, P).T).astype(np.float32)
    b2f = b2.reshape(1, P).T.astype(np.float32).copy()

    xhT = np.ascontiguousarray(xh_pad.T).astype(ml_dtypes.bfloat16).reshape(2, P, NP)

    in_maps = []
    for c in range(CORES):
        rows = slice(c * RPC, (c + 1) * RPC)
        dv = dis_pad[rows]
        atc = np.ascontiguousarray(
            at8[:, rows].reshape(KT, P, RPC).transpose(1, 0, 2).reshape(P, KT * RPC))
        m = {
            "at": atc,
            "xhT": xhT,
            "xoT": np.ascontiguousarray(x_pad[rows].T).astype(
                ml_dtypes.bfloat16).reshape(2, P, RPC),
            "w1x": w1x,
            "w2x": w2x,
            "b1f": b1f,
            "b2f": b2f,
            "ndisb": np.broadcast_to(-dv, (P, RPC)).copy(),
            "nd2b": np.broadcast_to(-(dv * dv), (P, RPC)).astype(ml_dtypes.bfloat16).copy(),
            "disb": np.broadcast_to(dv, (P, RPC)).astype(ml_dtypes.bfloat16).copy(),
        }
        in_maps.append(m)
    return in_maps


def _run(in_maps, trace=False, **kw):
    if "nc" not in _STATE:
        _STATE["nc"] = _build()
    r = run_bass_kernel_spmd(_STATE["nc"], in_maps, core_ids=list(range(CORES)),
                             trace=trace, **kw)
    out = np.concatenate([r.results[c]["outo"] for c in range(CORES)], axis=0)
    return out[:N], r


def kernel(**inputs) -> np.ndarray:
    in_maps = _prepare_inputs(**inputs)
    out, _ = _run(in_maps)
    return out


# revision 10
# speedup vs baseline: 1.1660x; 1.1660x over previous
"""ChebyNet (K=3, 2 layers) forward on 8 Trainium2 NeuronCores.

Node-sharded dense formulation. The sparse propagation L = -D^-1/2 A D^-1/2
runs as dense matmuls against SBUF-resident transposed adjacency counts
AT[s, d] (fp8e4m3, exact small ints), pair-interleaved per dst-chunk so each
DoubleRow matmul reads a contiguous [2 x w] moving block. All four hops run
"transposed" (feature tile stationary, adjacency moving) producing
feature-major [feat, dst] PSUM, chunk-major: each 512-wide dst chunk
completes its full accumulation, is post-processed, staged node-major, and
its AllGather piece fires while the next chunk is still sweeping.

Precision plan: features are fp8 with optional compensation. A hop with
parts=(hi,) sweeps once with MatmulPerfMode.DoubleRow (2 k-tiles/matmul at
1 cy/row = half the bf16 cost); parts=(hi,lo) adds the exact-residual sweep
(same total cost as bf16 but the AG's hi piece gates the consumer at half
the bytes). Hops A/C run hi-only (their terms are ~half the signal), hops
B/D compensated.

Layer algebra (dis-scaling folded into host-side x_hat = dis*x):
  u0    = x_hat @ (2 W12)                      (replicated, all NP rows)
  hop A: PA^T = AT-sweep(u0)                   s1_hat = d1_hat^T + nd2*PA^T
  AG0(s1_hat)
  hop B: PB^T = AT-sweep(s1_hat)               h^T = relu(e0^T + ndis*PB^T + b1)
  z1_hat^T, z2_hat^T from h_hat^T; hw^T from h^T  (W2 stationary)
  AG1(z2_hat)
  hop C: PC^T = AT-sweep(z2_hat)               s2_hat = z1_hat^T + nd2*PC^T
  AG2(s2_hat)
  hop D: PD^T = AT-sweep(s2_hat)               out^T = hw^T + ndis*PD^T + b2
"""

import sys

for _p in ("/opt/trn_rl_repo", "/root/.axon_site", "/root/.axon_site/_ro/trn_rl_repo",
           "/root/.axon_site/_ro/pypackages"):
    if _p not in sys.path:
        sys.path.append(_p)

import numpy as np
import ml_dtypes

import concourse.bacc as bacc
import concourse.tile as tile
from concourse import bass, mybir
from concourse.bass_utils import run_bass_kernel_spmd
from concourse.masks import make_identity

# problem constants (hardcoded per harness contract)
N, E, IN, HID, OUT, K = 10000, 320000, 256, 256, 128, 3
CORES = 8
NP, P = 10240, 128
RPC = NP // CORES           # rows per core = 1280
MB = RPC // P               # 128-blocks per core = 10
KT = NP // P                # src k-tiles = 80
PAIRS = KT // 2             # DoubleRow k-tile pairs = 40
F = IN                      # layer-1 prop width = 256
CHK = ((0, 512), (512, 512), (1024, 256))    # dst chunks == AG pieces
PIECES = ((0, 4), (4, 8), (8, 10))           # piece block ranges (chunk aligned)
AGW = (F, OUT, OUT)                          # AG widths per round
# sweep parts per hop: (0,) = fp8 hi only, (0,1) = compensated hi+lo
HOP_PARTS = {"A": (0,), "B": (0, 1), "C": (0,), "D": (0, 1)}

FP8 = mybir.dt.float8e4
BF16 = mybir.dt.bfloat16
F32 = mybir.dt.float32
DR = mybir.MatmulPerfMode.DoubleRow

_STATE = {}


def _piece_pair_sets():
    """Pairs j grouped by AG piece: piece blocks [b0,b1) -> j%(MB//2) in [b0/2,b1/2)."""
    sets = []
    for b0, b1 in PIECES:
        lo, hi = b0 // 2, b1 // 2
        sets.append([j for j in range(PAIRS) if lo <= (j % (MB // 2)) < hi])
    return sets


def _build():
    nc = bacc.Bacc("TRN2", target_bir_lowering=False, debug=False, num_devices=CORES)

    # adjacency, pair-interleaved per dst chunk: atc[ci][p, j, k, col]
    atc_d = [nc.dram_tensor(f"at{ci}", [P, PAIRS, 2, w], FP8, kind="ExternalInput")
             for ci, (off, w) in enumerate(CHK)]
    xhT_d = nc.dram_tensor("xhT", [2, P, NP], BF16, kind="ExternalInput")
    xoT_d = nc.dram_tensor("xoT", [2, P, RPC], BF16, kind="ExternalInput")
    w1x_d = nc.dram_tensor("w1x", [K, IN, HID], BF16, kind="ExternalInput")
    w2x_d = nc.dram_tensor("w2x", [K, HID, OUT], BF16, kind="ExternalInput")
    b1f_d = nc.dram_tensor("b1f", [P, 2], F32, kind="ExternalInput")
    b2f_d = nc.dram_tensor("b2f", [P, 1], F32, kind="ExternalInput")
    ndisb_d = nc.dram_tensor("ndisb", [P, RPC], BF16, kind="ExternalInput")
    nd2b_d = nc.dram_tensor("nd2b", [P, RPC], BF16, kind="ExternalInput")
    disb_d = nc.dram_tensor("disb", [P, RPC], BF16, kind="ExternalInput")
    out_d = nc.dram_tensor("outo", [RPC, OUT], F32, kind="ExternalOutput")

    psets = _piece_pair_sets()
    order_nat = list(range(PAIRS))
    order_pc = [j for s in psets for j in s]

    with tile.TileContext(nc) as tc:
        with (
            tc.tile_pool(name="res", bufs=1) as res,
            tc.tile_pool(name="wrk", bufs=1) as wrk,
            tc.tile_pool(name="pacc", bufs=1, space="PSUM") as pacc,
            tc.tile_pool(name="pterm", bufs=1, space="PSUM") as pterm,
            tc.tile_pool(name="ptr", bufs=1, space="PSUM") as ptr,
            tc.tile_pool(name="dram", bufs=1, space="DRAM") as dram,
        ):
            # ---- dummy AG first: absorb collective bootstrap ----
            dumi = dram.tile([P, 16], BF16, name="dumi")
            dumo = dram.tile([CORES * P, 16], BF16, name="dumo", addr_space="Shared")
            nc.sync.dma_start(dumi[:], xhT_d[0, :, 0:16])
            nc.gpsimd.collective_compute(
                "AllGather", mybir.AluOpType.bypass,
                replica_groups=[list(range(CORES))],
                ins=[dumi[:].opt()], outs=[dumo[:].opt()],
            )

            # ---- w1[2] first (d2 needs it immediately) ----
            w1t = [[None, None] for _ in range(K)]
            w2t = [[None, None] for _ in range(K)]
            for ih in range(2):
                t = res.tile([P, HID], BF16, tag=f"w1_2_{ih}", name=f"w1_2_{ih}")
                nc.sync.dma_start(t[:], w1x_d[2, ih * P:(ih + 1) * P, :])
                w1t[2][ih] = t

            ident = res.tile([P, P], F32, name="ident")
            make_identity(nc, ident[:])
            idb = res.tile([P, P], BF16, name="idb")
            nc.vector.tensor_copy(idb[:], ident[:])

            # ---- adjacency stream: at0+at1 on scalar queue, at2 later on sync
            at_t = [res.tile([P, PAIRS, 2, w], FP8, tag=f"at{ci}", name=f"at{ci}")
                    for ci, (off, w) in enumerate(CHK)]
            for ci in (0, 1):
                for h in range(2):
                    sl = slice(h * (PAIRS // 2), (h + 1) * (PAIRS // 2))
                    nc.scalar.dma_start(at_t[ci][:, sl, :, :], atc_d[ci][:, sl, :, :])

            # ---- replicated u0 = x_hat @ (2 W12): W stationary, x_hat moving,
            # feature-major PSUM, PE-transposed into node-major fp8 u tiles.
            u_hi = res.tile([P, KT, F], FP8, name="u_hi")
            u_lo = res.tile([P, KT, F], FP8, name="u_lo")
            A_LO = len(HOP_PARTS["A"]) > 1
            NCH = 20
            for nci in range(NCH):
                xc = [wrk.tile([P, 512], BF16, tag=f"xc{ih}", bufs=3,
                               name=f"xc{nci}_{ih}") for ih in range(2)]
                for ih in range(2):
                    nc.sync.dma_start(xc[ih][:],
                                      xhT_d[ih, :, nci * 512:(nci + 1) * 512])
                for hh in range(2):
                    pd = pterm.tile([P, 512], F32, tag="pt", bufs=2,
                                    name=f"d2_{nci}_{hh}")
                    for ih in range(2):
                        nc.tensor.matmul(pd[:], w1t[2][ih][:, hh * P:(hh + 1) * P],
                                         xc[ih][:], start=(ih == 0), stop=(ih == 1))
                    d2c = wrk.tile([P, 512], BF16, tag=f"d2c{hh}", bufs=2,
                                   name=f"d2c_{nci}_{hh}")
                    nc.vector.tensor_copy(d2c[:], pd[:])
                    for mi in range(4):
                        kt = nci * 4 + mi
                        tp = ptr.tile([P, P], BF16, tag="ptr", bufs=2,
                                      name=f"d2t_{kt}_{hh}")
                        nc.tensor.transpose(tp[:], d2c[:, mi * P:(mi + 1) * P], idb[:])
                        nc.vector.tensor_copy(u_hi[:, kt, hh * P:(hh + 1) * P], tp[:])
                        if A_LO:
                            nc.vector.tensor_sub(u_lo[:, kt, hh * P:(hh + 1) * P],
                                                 tp[:],
                                                 u_hi[:, kt, hh * P:(hh + 1) * P])

            # ---- remaining small loads on sync (needed from post-A onward) ----
            for k in range(K):
                for ih in range(2):
                    if w1t[k][ih] is None:
                        t = res.tile([P, HID], BF16, tag=f"w1_{k}_{ih}",
                                     name=f"w1_{k}_{ih}")
                        nc.sync.dma_start(t[:], w1x_d[k, ih * P:(ih + 1) * P, :])
                        w1t[k][ih] = t
                    t2 = res.tile([P, OUT], BF16, tag=f"w2_{k}_{ih}",
                                  name=f"w2_{k}_{ih}")
                    nc.sync.dma_start(t2[:], w2x_d[k, ih * P:(ih + 1) * P, :])
                    w2t[k][ih] = t2
            b1f = res.tile([P, 2], F32, name="b1f")
            nc.sync.dma_start(b1f[:], b1f_d[:])
            b2f = res.tile([P, 1], F32, name="b2f")
            nc.sync.dma_start(b2f[:], b2f_d[:])
            ndisb = res.tile([P, RPC], BF16, name="ndisb")
            nc.sync.dma_start(ndisb[:], ndisb_d[:])
            nd2b = res.tile([P, RPC], BF16, name="nd2b")
            nc.sync.dma_start(nd2b[:], nd2b_d[:])
            disb = res.tile([P, RPC], BF16, name="disb")
            nc.sync.dma_start(disb[:], disb_d[:])
            xoT = []
            for ih in range(2):
                t = res.tile([P, RPC], BF16, tag=f"xoT{ih}", name=f"xoT{ih}")
                nc.sync.dma_start(t[:], xoT_d[ih])
                xoT.append(t)
            # at2 tail on sync
            for h in range(2):
                sl = slice(h * (PAIRS // 2), (h + 1) * (PAIRS // 2))
                nc.sync.dma_start(at_t[2][:, sl, :, :], atc_d[2][:, sl, :, :])

            # ---- d1_hat^T, e0^T (feature-major, W stationary, own rows) ----
            d1T = [res.tile([P, RPC], BF16, tag=f"d1T{c}", name=f"d1T{c}")
                   for c in range(2)]
            e0T = [res.tile([P, RPC], BF16, tag=f"e0T{c}", name=f"e0T{c}")
                   for c in range(2)]
            for dst, wk, scaled in ((d1T, 1, True), (e0T, 0, False)):
                for hh in range(2):
                    for off, w in CHK:
                        pd = pterm.tile([P, 512], F32, tag="pt", bufs=2,
                                        name=f"f{wk}_{hh}_{off}")
                        for ih in range(2):
                            nc.tensor.matmul(
                                pd[:, :w], w1t[wk][ih][:, hh * P:(hh + 1) * P],
                                xoT[ih][:, off:off + w],
                                start=(ih == 0), stop=(ih == 1))
                        if scaled:
                            nc.vector.tensor_mul(dst[hh][:, off:off + w], pd[:, :w],
                                                 disb[:, off:off + w])
                        else:
                            nc.vector.tensor_copy(dst[hh][:, off:off + w], pd[:, :w])

            # ---- staging / AG machinery ----
            sgN_hi = res.tile([P, MB, F], FP8, name="sgN_hi")
            sgN_lo = res.tile([P, MB, F], FP8, name="sgN_lo")
            sA = [wrk.tile([P, RPC], BF16, tag=f"sA{c}", name=f"sA{c}")
                  for c in range(2)]

            rparts = [len(HOP_PARTS[h]) for h in "BCD"]  # parts shipped per round
            ag_in = [[[dram.tile([(b1 - b0) * P, AGW[r]], FP8,
                                 name=f"agi{r}_{part}_{pc}")
                       for pc, (b0, b1) in enumerate(PIECES)]
                      for part in range(rparts[r])] for r in range(3)]
            ag_out = [[[dram.tile([CORES * (b1 - b0) * P, AGW[r]], FP8,
                                  name=f"ago{r}_{part}_{pc}", addr_space="Shared")
                        for pc, (b0, b1) in enumerate(PIECES)]
                       for part in range(rparts[r])] for r in range(3)]

            def stage_block(r, mb, c, src):
                tp = ptr.tile([P, P], BF16, tag="ptr", bufs=2, name=f"tp{r}_{mb}_{c}")
                nc.tensor.transpose(tp[:], src, idb[:])
                nc.vector.tensor_copy(sgN_hi[:, mb, c * P:c * P + P], tp[:])
                if rparts[r] > 1:
                    nc.vector.tensor_sub(sgN_lo[:, mb, c * P:c * P + P], tp[:],
                                         sgN_hi[:, mb, c * P:c * P + P])

            def emit_ag(r, part, pc):
                b0, b1 = PIECES[pc]
                sg = sgN_hi if part == 0 else sgN_lo
                nc.sync.dma_start(
                    ag_in[r][part][pc][:].rearrange("(b p) f -> p b f", p=P),
                    sg[:, b0:b1, :AGW[r]])
                nc.gpsimd.collective_compute(
                    "AllGather", mybir.AluOpType.bypass,
                    replica_groups=[list(range(CORES))],
                    ins=[ag_in[r][part][pc][:].opt()],
                    outs=[ag_out[r][part][pc][:].opt()],
                )

            def reload(r, part, pc):
                b0, b1 = PIECES[pc]
                nb = b1 - b0
                ut = u_hi if part == 0 else u_lo
                for c in range(CORES):
                    src = ag_out[r][part][pc][c * nb * P:(c + 1) * nb * P, :] \
                        .rearrange("(b p) f -> p b f", p=P)
                    eng = nc.sync if c % 2 == 0 else nc.scalar
                    eng.dma_start(ut[:, c * MB + b0:c * MB + b0 + nb, :AGW[r]], src)

            def stage_chunk(r, ci, nchalf, zsrc):
                """Stage this chunk's blocks node-major and fire its hi AG piece.
                Reloads are deferred to finish_round: they write u tiles the
                producing sweep is still reading, so issuing them inline would
                block the in-order DMA queues on the sweep's completion."""
                b0, b1 = PIECES[ci]
                for mb in range(b0, b1):
                    for c in range(nchalf):
                        stage_block(r, mb, c, zsrc[c][:, mb * P:(mb + 1) * P])
                emit_ag(r, 0, ci)

            def finish_round(r):
                for pc in range(len(PIECES)):
                    reload(r, 0, pc)
                if rparts[r] > 1:
                    for pc in range(len(PIECES)):
                        emit_ag(r, 1, pc)
                    for pc in range(len(PIECES)):
                        reload(r, 1, pc)

            def sweep(hop, order, nchalf, post_chunk):
                parts = [(u_hi, u_lo)[i] for i in HOP_PARTS[hop]]
                for ci, (off, w) in enumerate(CHK):
                    paps = [pacc.tile([P, 512], F32, tag=f"pa{c}", bufs=2,
                                      name=f"sw{hop}_{ci}_{c}")
                            for c in range(nchalf)]
                    for pi, ut in enumerate(parts):
                        for c in range(nchalf):
                            for jn, j in enumerate(order):
                                first = (pi == 0 and c == 0 and jn == 0)
                                last = (pi == len(parts) - 1 and c == nchalf - 1
                                        and jn == len(order) - 1)
                                nc.tensor.matmul(
                                    paps[c][:, :w],
                                    ut[:, 2 * j:2 * j + 2, c * P:c * P + P],
                                    at_t[ci][:, j, :, :],
                                    start=(pi == 0 and jn == 0),
                                    stop=(pi == len(parts) - 1
                                          and jn == len(order) - 1),
                                    perf_mode=DR)
                    post_chunk(ci, off, w, paps)

            # ================= hop A =================
            def postA(ci, off, w, paps):
                for c in range(2):
                    nc.vector.tensor_mul(sA[c][:, off:off + w], paps[c][:, :w],
                                         nd2b[:, off:off + w])
                    nc.vector.tensor_add(sA[c][:, off:off + w], sA[c][:, off:off + w],
                                         d1T[c][:, off:off + w])
                stage_chunk(0, ci, 2, sA)

            sweep("A", order_nat, 2, postA)
            finish_round(0)

            # ================= hop B =================
            hb = [res.tile([P, RPC], BF16, tag=f"hb{c}", name=f"hb{c}")
                  for c in range(2)]
            hhb = [res.tile([P, RPC], BF16, tag=f"hhb{c}", name=f"hhb{c}")
                   for c in range(2)]
            z1T = res.tile([P, RPC], BF16, tag="z1T", name="z1T")
            hwT = res.tile([P, RPC], BF16, tag="hwT", name="hwT")

            def postB(ci, off, w, paps):
                for c in range(2):
                    t = wrk.tile([P, 512], F32, tag="t32", bufs=2,
                                 name=f"t32_{ci}_{c}")
                    nc.vector.tensor_mul(t[:, :w], paps[c][:, :w],
                                         ndisb[:, off:off + w])
                    nc.vector.tensor_add(t[:, :w], t[:, :w], e0T[c][:, off:off + w])
                    nc.vector.tensor_scalar_add(t[:, :w], t[:, :w], b1f[:, c:c + 1])
                    nc.vector.tensor_scalar_max(hb[c][:, off:off + w], t[:, :w], 0.0)
                    nc.gpsimd.tensor_mul(hhb[c][:, off:off + w],
                                         hb[c][:, off:off + w], disb[:, off:off + w])
                for wk, mov, dst in ((2, hhb, sA[0]), (1, hhb, z1T), (0, hb, hwT)):
                    pz = pterm.tile([P, 512], F32, tag="pt", bufs=2,
                                    name=f"z{wk}_{ci}")
                    for ih in range(2):
                        nc.tensor.matmul(pz[:, :w], w2t[wk][ih][:],
                                         mov[ih][:, off:off + w],
                                         start=(ih == 0), stop=(ih == 1))
                    nc.vector.tensor_copy(dst[:, off:off + w], pz[:, :w])
                stage_chunk(1, ci, 1, sA)

            sweep("B", order_pc, 2, postB)
            finish_round(1)

            # ================= hop C =================
            def postC(ci, off, w, paps):
                nc.vector.tensor_mul(sA[0][:, off:off + w], paps[0][:, :w],
                                     nd2b[:, off:off + w])
                nc.vector.tensor_add(sA[0][:, off:off + w], sA[0][:, off:off + w],
                                     z1T[:, off:off + w])
                stage_chunk(2, ci, 1, sA)

            sweep("C", order_pc, 1, postC)
            finish_round(2)

            # ================= hop D =================
            def postD(ci, off, w, paps):
                t = wrk.tile([P, 512], F32, tag="t32", bufs=2, name=f"to_{ci}")
                nc.vector.tensor_mul(t[:, :w], paps[0][:, :w], ndisb[:, off:off + w])
                nc.vector.tensor_add(t[:, :w], t[:, :w], hwT[:, off:off + w])
                nc.vector.tensor_scalar_add(t[:, :w], t[:, :w], b2f[:, 0:1])
                for mi in range(w // P):
                    mb = off // P + mi
                    tb = wrk.tile([P, P], BF16, tag="tb", bufs=2, name=f"tb_{mb}")
                    nc.vector.tensor_copy(tb[:], t[:, mi * P:(mi + 1) * P])
                    tf = ptr.tile([P, P], BF16, tag="ptr", bufs=2, name=f"tf_{mb}")
                    nc.tensor.transpose(tf[:], tb[:], idb[:])
                    ob = wrk.tile([P, OUT], F32, tag="ob", bufs=1, name=f"ob_{mb}")
                    nc.vector.tensor_copy(ob[:], tf[:])
                    nc.sync.dma_start(out_d[mb * P:(mb + 1) * P, :], ob[:])

            sweep("D", order_pc, 1, postD)

    nc.compile()
    return nc


def _prepare_inputs(x, edge, W1, b1, W2, b2):
    x = np.asarray(x, np.float32)
    edge = np.asarray(edge)
    W1 = np.asarray(W1, np.float32)
    b1 = np.asarray(b1, np.float32)
    W2 = np.asarray(W2, np.float32)
    b2 = np.asarray(b2, np.float32)
    src = edge[0].astype(np.int64)
    dst = edge[1].astype(np.int64)

    deg = np.bincount(dst, minlength=N).astype(np.float32)
    dis = np.where(deg > 0, 1.0 / np.sqrt(np.maximum(deg, 1.0)), 0.0).astype(np.float32)

    # dense transposed adjacency counts AT[s, d] (exact in fp8)
    flat = src * NP + dst
    uniq, cnt = np.unique(flat, return_counts=True)
    at8 = np.zeros(NP * NP, dtype=ml_dtypes.float8_e4m3)
    at8[uniq] = cnt.astype(ml_dtypes.float8_e4m3)
    at8 = at8.reshape(NP, NP)

    dis_pad = np.zeros(NP, np.float32)
    dis_pad[:N] = dis
    x_pad = np.zeros((NP, F), np.float32)
    x_pad[:N] = x
    xh_pad = x_pad * dis_pad[:, None]

    w1x = np.stack([W1[0] - W1[2], W1[1], 2.0 * W1[2]]).astype(ml_dtypes.bfloat16)
    w2x = np.stack([W2[0] - W2[2], W2[1], 2.0 * W2[2]]).astype(ml_dtypes.bfloat16)
    b1f = np.ascontiguousarray(b1.reshape(2, P).T).astype(np.float32)
    b2f = b2.reshape(1, P).T.astype(np.float32).copy()

    xhT = np.ascontiguousarray(xh_pad.T).astype(ml_dtypes.bfloat16).reshape(2, P, NP)

    in_maps = []
    for c in range(CORES):
        rows = slice(c * RPC, (c + 1) * RPC)
        dv = dis_pad[rows]
        atcore = at8[:, rows]  # [NP, RPC]
        m = {
            "xhT": xhT,
            "xoT": np.ascontiguousarray(x_pad[rows].T).astype(
                ml_dtypes.bfloat16).reshape(2, P, RPC),
            "w1x": w1x,
            "w2x": w2x,
            "b1f": b1f,
            "b2f": b2f,
            "ndisb": np.broadcast_to(-dv, (P, RPC)).astype(ml_dtypes.bfloat16).copy(),
            "nd2b": np.broadcast_to(-(dv * dv), (P, RPC)).astype(
                ml_dtypes.bfloat16).copy(),
            "disb": np.broadcast_to(dv, (P, RPC)).astype(ml_dtypes.bfloat16).copy(),
        }
        for ci, (off, w) in enumerate(CHK):
            slab = atcore[:, off:off + w]                     # [NP, w]
            m[f"at{ci}"] = np.ascontiguousarray(
                slab.reshape(PAIRS, 2, P, w).transpose(2, 0, 1, 3))
        in_maps.append(m)
    return in_maps


def _run(in_maps, trace=False, **kw):
    if "nc" not in _STATE:
        _STATE["nc"] = _build()
    r = run_bass_kernel_spmd(_STATE["nc"], in_maps, core_ids=list(range(CORES)),
                             trace=trace, **kw)
    out = np.concatenate([r.results[c]["outo"] for c in range(CORES)], axis=0)
    return out[:N], r


def kernel(**inputs) -> np.ndarray:
    in_maps = _prepare_inputs(**inputs)
    out, _ = _run(in_maps)
    return out
